# revision 1
# baseline (speedup 1.0000x reference)
"""Trainium2 Bass kernel for nn_DelayGSCSNN.

Two-layer adaptive-LIF spiking net with learnable input delays, BN (eval),
and a leaky-integrator readout, scanned over T=100 steps.

Strategy (data-parallel over batch, 8 cores, no collectives):
  - each core simulates B/8 = 32 samples; weights replicated in SBUF.
  - on-device layout: neurons on partitions, batch on the free dim, so the
    spike tiles s1T [128, 8*32] / s2T [128, 4*32] are directly the matmul
    moving operands (rhs) for the next step -- no transposes anywhere.
  - host folds BN (eval stats) and the (1-alpha) input scaling into the
    weight matrices; folds the -THRESH*s and -beta_a*s terms into the
    recurrent weight diagonal; tracks adaptation as A = -a/beta_a so each
    layer's state update is 4 fused DVE ops per step.
  - per-channel fractional delays are folded into a lag-grouped input
    projection (one matmul per step over K = n_lags*C + 1 incl. bias row);
    the lag shift itself is a column-offset copy done once in the prologue.
  - readout: acc = sum_t (1-beta^(T-t))/T * s2_t @ W_out.T, accumulated as
    a weighted spike sum (one DVE op/step) and one matmul at the end.
  - weights in bf16 (spikes are exactly representable; matmul accumulates
    in fp32), state updates in fp32 on the vector engine.
"""

import os
import sys

import numpy as np

for _p in ("/opt/trn_rl_repo", "/root/.axon_site/_ro/trn_rl_repo"):
    if os.path.isdir(_p) and _p not in sys.path:
        sys.path.insert(0, _p)

import concourse.bass as bass
import concourse.tile as tile
from concourse import bacc, mybir
from concourse.bass_utils import run_bass_kernel_spmd

import ml_dtypes

F32 = mybir.dt.float32
BF16 = mybir.dt.bfloat16
OP = mybir.AluOpType

B, T, C = 256, 100, 40
H1, H2, O = 1024, 512, 35
THRESH = 1.0
MAX_DELAY = 30
NCORES = 8
BC = B // NCORES  # batch per core = 32

TRACE = False
TMPDIR = None
LAST_RESULT = None

_CACHE = {}


def _uniform(v):
    v = np.asarray(v, np.float64)
    return float(v.flat[0]) if np.ptp(v) == 0 else None


FP8 = os.environ.get("KFP8", "0") == "1"


def _build_bass(n_lags, lags, kin, scal):
    """Build the Bass program. scal: dict of python-float uniform params."""
    WD = mybir.dt.float8e4 if FP8 else BF16
    nc = bacc.Bacc(None, target_bir_lowering=False)

    # DRAM inputs (per-core shapes; host supplies prepared layouts)
    d_lt1 = nc.dram_tensor("lt1", [128, 8 * H1], WD, kind="ExternalInput")
    d_lt2 = nc.dram_tensor("lt2", [128, 8 * H2], WD, kind="ExternalInput")
    d_ltr2 = nc.dram_tensor("ltr2", [128, 4 * H2], WD, kind="ExternalInput")
    d_lto = nc.dram_tensor("lto", [128, 4 * O], F32, kind="ExternalInput")
    d_wcat = nc.dram_tensor("wcat", [128, H1], BF16, kind="ExternalInput")
    d_x = nc.dram_tensor("xcore", [C, T, BC], F32, kind="ExternalInput")
    d_y = nc.dram_tensor("yout", [O, BC], F32, kind="ExternalOutput")

    A1 = scal["al1"]
    R1 = scal["rh1"]
    RBA1 = scal["rh1"] * scal["ba1"]
    A2 = scal["al2"]
    R2 = scal["rh2"]
    RBA2 = scal["rh2"] * scal["ba2"]
    wt = scal["wt"]  # list of T readout weights (1-beta^(T-t))/T

    with tile.TileContext(nc) as tc:
        with (
            tc.tile_pool(name="const", bufs=1) as cpool,
            tc.tile_pool(name="state", bufs=1) as spool,
            tc.tile_pool(name="psum", bufs=2, space="PSUM") as ppool,
            tc.tile_pool(name="pout", bufs=1, space="PSUM") as opool,
        ):
            # --- weights / input staging ---
            lt1 = cpool.tile([128, 8 * H1], WD)
            lt2 = cpool.tile([128, 8 * H2], WD)
            ltr2 = cpool.tile([128, 4 * H2], WD)
            lto = cpool.tile([128, 4 * O], F32)
            wcat = cpool.tile([128, H1], BF16)
            xf32 = cpool.tile([C, T * BC], F32)
            xt = cpool.tile([128, T * BC], BF16)

            nc.sync.dma_start(lt1[:], d_lt1[:])
            nc.sync.dma_start(lt2[:], d_lt2[:])
            nc.sync.dma_start(ltr2[:], d_ltr2[:])
            nc.sync.dma_start(lto[:], d_lto[:])
            nc.sync.dma_start(wcat[:], d_wcat[:])
            nc.sync.dma_start(xf32[:], d_x[:].rearrange("c t b -> c (t b)"))

            # Engine writes must start at a 32-aligned partition: lag blocks
            # live at partitions 0 and 64; the constant-one bias row is carved
            # from a 32-aligned ones fill (lag1's copy overwrites part of it,
            # leaving row `bias_row` = 1.0 with zero rows after it).
            ones_base = 64 if n_lags == 1 else 96
            salt = len(os.environ.get("KSALT", ""))
            if salt:
                # compile-cache salt: harmless extra memset changes the BIR
                # hash so A/B compiler-flag experiments don't hit the cache
                sc = cpool.tile([1, salt], F32)
                nc.vector.memset(sc[:], 0.0)
            nc.vector.memset(xt[:], 0.0)
            nc.vector.memset(xt[ones_base:128, :], 1.0)
            for li, lg in enumerate(lags):
                if lg < T:
                    nc.vector.tensor_copy(
                        xt[64 * li : 64 * li + C, lg * BC : T * BC],
                        xf32[:, 0 : (T - lg) * BC],
                    )

            # --- states ---
            v1 = spool.tile([128, 8 * BC], F32)
            a1n = spool.tile([128, 8 * BC], F32)
            v2 = spool.tile([128, 4 * BC], F32)
            a2n = spool.tile([128, 4 * BC], F32)
            s2w = spool.tile([128, 4 * BC], F32)
            # spike tiles are parity double-buffered: step t writes buffer
            # t%2 and reads buffer (t-1)%2, so the adaptation update (which
            # reads the OLD spikes) can be emitted AFTER the spike write --
            # keeping the DVE FIFO ahead of the spike write short, since s1
            # gates 96 matmuls of the next round.
            s1b = [spool.tile([128, 8 * BC], BF16, name=f"s1_{i}") for i in range(2)]
            s2b = [spool.tile([128, 4 * BC], BF16, name=f"s2_{i}") for i in range(2)]
            for st in (v1, a1n, v2, a2n, s2w):
                nc.vector.memset(st[:], 0.0)
            for st in s1b + s2b:
                nc.vector.memset(st[:], 0.0)
            if FP8:
                # separate fp8 copies of the spike tiles feed the matmuls
                # (0/1 is exact in fp8); the DVE state update reads bf16.
                s1qb = [spool.tile([128, 8 * BC], WD, name=f"s1q_{i}") for i in range(2)]
                s2qb = [spool.tile([128, 4 * BC], WD, name=f"s2q_{i}") for i in range(2)]
                for st in s1qb + s2qb:
                    nc.vector.memset(st[:], 0.0)
            else:
                s1qb, s2qb = s1b, s2b

            mul, add, sub = OP.mult, OP.add, OP.subtract

            def input_mms(t):
                # input projection into its own PSUM tile as 8 self-contained
                # per-mt groups (start+stop each), so it can be issued one
                # step ahead -- it depends only on xt, and fills the PE stall
                # while the DVE computes the layer-1 spikes.
                pi = ppool.tile([128, 8 * BC], F32, tag="pin")
                for mt in range(8):
                    nc.tensor.matmul(
                        pi[:, mt * BC : (mt + 1) * BC],
                        wcat[0:kin, mt * 128 : (mt + 1) * 128],
                        xt[0:kin, t * BC : (t + 1) * BC],
                        start=True,
                        stop=True,
                    )
                return pi

            pin_next = input_mms(0)

            for t in range(T):
                w, r = t % 2, (t - 1) % 2
                s1, s2 = s1b[w], s2b[w]
                s1o, s2o = s1b[r], s2b[r]
                s1q, s2q = s1qb[w], s2qb[w]
                s1qo, s2qo = s1qb[r], s2qb[r]
                # ---- PE: psum1 = LT1eff @ s1_{t-1} (input part was issued
                # one step ahead into pin_next) ----
                pin = pin_next
                p1 = None
                if t > 0:
                    p1 = ppool.tile([128, 8 * BC], F32, tag="p1")
                    for mt in range(8):
                        po = p1[:, mt * BC : (mt + 1) * BC]
                        for kt in range(8):
                            nc.tensor.matmul(
                                po,
                                lt1[:, kt * H1 + mt * 128 : kt * H1 + (mt + 1) * 128],
                                s1qo[:, kt * BC : (kt + 1) * BC],
                                start=(kt == 0),
                                stop=(kt == 7),
                            )

                p2 = ppool.tile([128, 4 * BC], F32, tag="p2")

                # ---- DVE: layer-1 state update ----
                # v1 = alpha*v1 + pin (runs during rec1) ; v1 += psum1
                # v1 += rho*beta_a*A1neg ; s1 = (v1 >= THRESH)
                # A1neg = rho*A1neg - s1_old (deferred behind the spike write)
                nc.vector.scalar_tensor_tensor(v1[:], v1[:], A1, pin[:], mul, add)
                nc.vector.scalar_tensor_tensor(v1[:], a1n[:], RBA1, v1[:], mul, add)
                if p1 is not None:
                    nc.vector.tensor_add(v1[:], v1[:], p1[:])
                nc.vector.tensor_scalar(s1[:], v1[:], THRESH, None, OP.is_ge)
                if FP8:
                    nc.vector.tensor_scalar(s1q[:], v1[:], THRESH, None, OP.is_ge)
                nc.vector.scalar_tensor_tensor(a1n[:], a1n[:], R1, s1o[:], mul, sub)

                # ---- PE: next step's input projection (independent of s1_t;
                # fills the stall while the DVE computes the spikes) ----
                if t + 1 < T:
                    pin_next = input_mms(t + 1)

                # ---- PE: psum2 = LTr2eff @ s2_{t-1} + LT2eff @ s1_t ----
                # per-mt accumulation groups must be contiguous (one pending
                # group per PSUM tile); rec2 leads so it can start before the
                # DVE finishes s1_t.
                for mt in range(4):
                    po = p2[:, mt * BC : (mt + 1) * BC]
                    if t > 0:
                        for kt in range(4):
                            nc.tensor.matmul(
                                po,
                                ltr2[:, kt * H2 + mt * 128 : kt * H2 + (mt + 1) * 128],
                                s2qo[:, kt * BC : (kt + 1) * BC],
                                start=(kt == 0),
                                stop=False,
                            )
                    for kt in range(8):
                        nc.tensor.matmul(
                            po,
                            lt2[:, kt * H2 + mt * 128 : kt * H2 + (mt + 1) * 128],
                            s1q[:, kt * BC : (kt + 1) * BC],
                            start=(kt == 0 and t == 0),
                            stop=(kt == 7),
                        )

                # ---- DVE: layer-2 state update + readout accumulation ----
                nc.vector.scalar_tensor_tensor(v2[:], v2[:], A2, p2[:], mul, add)
                nc.vector.scalar_tensor_tensor(v2[:], a2n[:], RBA2, v2[:], mul, add)
                nc.vector.tensor_scalar(s2[:], v2[:], THRESH, None, OP.is_ge)
                if FP8:
                    nc.vector.tensor_scalar(s2q[:], v2[:], THRESH, None, OP.is_ge)
                nc.vector.scalar_tensor_tensor(a2n[:], a2n[:], R2, s2o[:], mul, sub)
                nc.vector.scalar_tensor_tensor(s2w[:], s2[:], wt[t], s2w[:], mul, add)

            # ---- epilogue: acc^T = W_out @ s2w  -> DMA out ----
            pO = opool.tile([O, BC], F32)
            # matmul needs matching dtypes; s2w is fp32, lto fp32 (4 cyc/row,
            # only 4 small matmuls).
            for kt in range(4):
                nc.tensor.matmul(
                    pO[:],
                    lto[:, kt * O : (kt + 1) * O],
                    s2w[:, kt * BC : (kt + 1) * BC],
                    start=(kt == 0),
                    stop=(kt == 3),
                )
            yt = spool.tile([O, BC], F32)
            nc.vector.tensor_copy(yt[:], pO[:])
            nc.sync.dma_start(d_y[:], yt[:])

    nc.finalize()
    return nc


# KV=2 selects the experimental PSUM-injection/lagged-layer-2 build below.
# Measured on hw it is neutral vs the v1 build (~548us vs ~540us): a
# ~1.4us/step cross-engine semaphore serialization (the layer-1 spike
# threshold op's PE-tick wait resolves only at the end of the whole
# iteration's PE stream) eats the pipelining win.  Default to the proven v1.
KV = os.environ.get("KV", "1")
KDR = os.environ.get("KDR", "0") == "1"  # fp8 DoubleRow for rec/ff matmuls
# (measured: DoubleRow LDWEIGHTS is ~3x slower per tile on this hw -- keep off)
KGPS = os.environ.get("KGPS", "0") == "1"  # adaptation/m updates on GpSimd
# (GpSimd lowers to the Pool engine on TRN2, which rejects TensorScalarPtr
# at codegen -- keep these on the DVE.)


def _build_v2(n_lags, lags, kin, scal):
    """v2: all linear state terms injected into the PSUM accumulation group
    (identity matmul on m = alpha*v + kRBA*chat), spike threshold is the only
    critical-path DVE op, layer-2 matmuls lag one step behind layer 1 in the
    PE stream so the PE never waits on the spike DVE op. ACT does the
    leak-scaled PSUM->SBUF copies, GpSimd the adaptation updates.
    Spikes stored as exact 0/1 (fp8 when KDR, else bf16)."""
    WD = mybir.dt.float8e4 if KDR else BF16
    nc = bacc.Bacc(None, target_bir_lowering=False)

    d_lt1 = nc.dram_tensor("lt1", [128, 8 * H1], WD, kind="ExternalInput")
    d_lt2 = nc.dram_tensor("lt2", [128, 8 * H2], WD, kind="ExternalInput")
    d_ltr2 = nc.dram_tensor("ltr2", [128, 4 * H2], WD, kind="ExternalInput")
    d_lto = nc.dram_tensor("lto", [128, 4 * O], F32, kind="ExternalInput")
    d_wcat = nc.dram_tensor("wcat", [128, H1], BF16, kind="ExternalInput")
    d_eye = nc.dram_tensor("eye", [128, 128], BF16, kind="ExternalInput")
    d_x = nc.dram_tensor("xcore", [C, T, BC], F32, kind="ExternalInput")
    d_y = nc.dram_tensor("yout", [O, BC], F32, kind="ExternalOutput")

    A1, R1, RBA1 = scal["al1"], scal["rh1"], scal["rh1"] * scal["ba1"]
    A2, R2, RBA2 = scal["al2"], scal["rh2"], scal["rh2"] * scal["ba2"]
    wt = scal["wt"]

    with tile.TileContext(nc) as tc:
        with (
            tc.tile_pool(name="const", bufs=1) as cpool,
            tc.tile_pool(name="state", bufs=1) as spool,
            tc.tile_pool(name="psum1", bufs=2, space="PSUM") as p1pool,
            tc.tile_pool(name="psum2", bufs=2, space="PSUM") as p2pool,
            tc.tile_pool(name="pout", bufs=1, space="PSUM") as opool,
        ):
            lt1 = cpool.tile([128, 8, H1], WD)
            lt2 = cpool.tile([128, 8, H2], WD)
            ltr2 = cpool.tile([128, 4, H2], WD)
            lto = cpool.tile([128, 4 * O], F32)
            wcat = cpool.tile([128, H1], BF16)
            eye = cpool.tile([128, 128], BF16)
            xf32 = cpool.tile([C, T * BC], F32)
            xt = cpool.tile([128, T * BC], BF16)

            nc.sync.dma_start(lt1[:], d_lt1[:].rearrange("p (k m) -> p k m", k=8))
            nc.sync.dma_start(lt2[:], d_lt2[:].rearrange("p (k m) -> p k m", k=8))
            nc.sync.dma_start(ltr2[:], d_ltr2[:].rearrange("p (k m) -> p k m", k=4))
            nc.sync.dma_start(lto[:], d_lto[:])
            nc.sync.dma_start(wcat[:], d_wcat[:])
            nc.sync.dma_start(eye[:], d_eye[:])
            nc.sync.dma_start(xf32[:], d_x[:].rearrange("c t b -> c (t b)"))

            ones_base = 64 if n_lags == 1 else 96
            nc.vector.memset(xt[:], 0.0)
            nc.vector.memset(xt[ones_base:128, :], 1.0)
            for li, lg in enumerate(lags):
                if lg < T:
                    nc.vector.tensor_copy(
                        xt[64 * li : 64 * li + C, lg * BC : T * BC],
                        xf32[:, 0 : (T - lg) * BC],
                    )

            # --- states (3D: [128, chunk, batch]) ---
            s1b = [spool.tile([128, 8, BC], WD, name=f"s1_{i}") for i in range(2)]
            s2b = [spool.tile([128, 4, BC], WD, name=f"s2_{i}") for i in range(2)]
            c1 = spool.tile([128, 8, BC], BF16)
            c2 = spool.tile([128, 4, BC], BF16)
            va1b = [spool.tile([128, 8, BC], BF16, name=f"va1_{i}") for i in range(2)]
            va2b = [spool.tile([128, 4, BC], BF16, name=f"va2_{i}") for i in range(2)]
            m1b = [spool.tile([128, 8, BC], BF16, name=f"m1_{i}") for i in range(2)]
            m2b = [spool.tile([128, 4, BC], BF16, name=f"m2_{i}") for i in range(2)]
            s2w = spool.tile([128, 4, BC], F32)
            for st in s1b + s2b + va1b + va2b + m1b + m2b + [c1, c2, s2w]:
                nc.vector.memset(st[:], 0.0)

            mul, add, sub = OP.mult, OP.add, OP.subtract
            aux = nc.gpsimd if KGPS else nc.vector

            def g1(i):
                # layer-1 accumulation group for step i into p1 (per-mt
                # contiguous groups: pin opens, rec1 accumulates, ident(m1)
                # closes).  p1 = pin(i) + W1eff@s1(i-1) + m1(i-1)
                # All states zero-initialized, so every step is uniform.
                p1 = p1pool.tile([128, 8, BC], F32, tag="p1")
                s1o = s1b[(i - 1) % 2]
                m1 = m1b[(i - 1) % 2]
                for mt in range(8):
                    po = p1[:, mt, :]
                    ms = slice(mt * 128, (mt + 1) * 128)
                    nc.tensor.matmul(
                        po,
                        wcat[0:kin, ms],
                        xt[0:kin, i * BC : (i + 1) * BC],
                        start=True,
                        stop=False,
                    )
                    if KDR:
                        for kp in range(4):
                            nc.tensor.matmul(
                                po, lt1[:, 2 * kp : 2 * kp + 2, ms],
                                s1o[:, 2 * kp : 2 * kp + 2, :],
                                start=False, stop=False,
                                perf_mode=mybir.MatmulPerfMode.DoubleRow,
                            )
                    else:
                        for kt in range(8):
                            nc.tensor.matmul(
                                po, lt1[:, kt, ms], s1o[:, kt, :],
                                start=False, stop=False,
                            )
                    nc.tensor.matmul(po, eye[:], m1[:, mt, :], start=False, stop=True)
                return p1

            def g2(j):
                # layer-2 group for step j: p2 = W2eff@s1(j) + Wr2eff@s2(j-1)
                # + m2(j-1)
                p2 = p2pool.tile([128, 4, BC], F32, tag="p2")
                s1c = s1b[j % 2]
                s2o = s2b[(j - 1) % 2]
                m2 = m2b[(j - 1) % 2]
                for mt in range(4):
                    po = p2[:, mt, :]
                    ms = slice(mt * 128, (mt + 1) * 128)
                    if KDR:
                        for kp in range(4):
                            nc.tensor.matmul(
                                po, lt2[:, 2 * kp : 2 * kp + 2, ms],
                                s1c[:, 2 * kp : 2 * kp + 2, :],
                                start=(kp == 0), stop=False,
                                perf_mode=mybir.MatmulPerfMode.DoubleRow,
                            )
                        for kp in range(2):
                            nc.tensor.matmul(
                                po, ltr2[:, 2 * kp : 2 * kp + 2, ms],
                                s2o[:, 2 * kp : 2 * kp + 2, :],
                                start=False, stop=False,
                                perf_mode=mybir.MatmulPerfMode.DoubleRow,
                            )
                    else:
                        for kt in range(8):
                            nc.tensor.matmul(
                                po, lt2[:, kt, ms], s1c[:, kt, :],
                                start=(kt == 0), stop=False,
                            )
                        for kt in range(4):
                            nc.tensor.matmul(
                                po, ltr2[:, kt, ms], s2o[:, kt, :],
                                start=False, stop=False,
                            )
                    nc.tensor.matmul(po, eye[:], m2[:, mt, :], start=False, stop=True)
                return p2

            Copy = mybir.ActivationFunctionType.Copy
            p2b = [None, None]
            for i in range(T):
                j = i - 1
                p1_new = g1(i)
                # DVE: layer-1 spikes for step i (the only critical-path op)
                # -- emitted immediately after its producing group so Tile's
                # tick-based waits release it as early as possible.
                nc.vector.tensor_scalar(
                    s1b[i % 2][:], p1_new[:], THRESH, None, OP.is_ge
                )
                # ACT: leak-scaled copy valpha1(i) = A1 * v1(i)
                nc.scalar.activation(va1b[i % 2][:], p1_new[:], Copy, scale=A1)

                # DVE: is_ge2 for step j-1 is emitted one iteration LATE
                # (before g2(j), which consumes its spikes): its conservative
                # PE sem wait is then already satisfied when it executes, so
                # the critical is_ge1 above never queues behind it.
                if i >= 2:
                    nc.vector.tensor_scalar(
                        s2b[(j - 1) % 2][:], p2b[(j - 1) % 2][:], THRESH, None,
                        OP.is_ge,
                    )
                p2_new = g2(j) if i > 0 else None
                if p2_new is not None:
                    nc.scalar.activation(va2b[j % 2][:], p2_new[:], Copy, scale=A2)
                    p2b[j % 2] = p2_new

                # DVE: adaptation + m updates (off critical path)
                if i >= 1:
                    # chat1(i) = R1*chat1(i-1) - s1(i-1)
                    aux.scalar_tensor_tensor(
                        c1[:], c1[:], R1, s1b[(i - 1) % 2][:], mul, sub
                    )
                # m1(i) = RBA1*chat1(i) + valpha1(i)
                aux.scalar_tensor_tensor(
                    m1b[i % 2][:], c1[:], RBA1, va1b[i % 2][:], mul, add
                )
                if p2_new is not None:
                    if j >= 1:
                        aux.scalar_tensor_tensor(
                            c2[:], c2[:], R2, s2b[(j - 1) % 2][:], mul, sub
                        )
                    aux.scalar_tensor_tensor(
                        m2b[j % 2][:], c2[:], RBA2, va2b[j % 2][:], mul, add
                    )
                    if j >= 1:
                        # readout accumulation for step j-1
                        aux.scalar_tensor_tensor(
                            s2w[:], s2b[(j - 1) % 2][:], wt[j - 1], s2w[:], mul, add
                        )

            # drain: spikes/readout for steps T-2 and T-1 of layer 2
            j = T - 1
            p2_new = g2(j)
            p2b[j % 2] = p2_new
            nc.vector.tensor_scalar(
                s2b[(j - 1) % 2][:], p2b[(j - 1) % 2][:], THRESH, None, OP.is_ge
            )
            aux.scalar_tensor_tensor(
                s2w[:], s2b[(j - 1) % 2][:], wt[j - 1], s2w[:], mul, add
            )
            nc.vector.tensor_scalar(s2b[j % 2][:], p2_new[:], THRESH, None, OP.is_ge)
            aux.scalar_tensor_tensor(s2w[:], s2b[j % 2][:], wt[j], s2w[:], mul, add)

            # ---- epilogue: acc^T = W_out @ s2w -> DMA out ----
            pO = opool.tile([O, BC], F32)
            for kt in range(4):
                nc.tensor.matmul(
                    pO[:],
                    lto[:, kt * O : (kt + 1) * O],
                    s2w[:, kt, :],
                    start=(kt == 0),
                    stop=(kt == 3),
                )
            yt = spool.tile([O, BC], F32)
            nc.vector.tensor_copy(yt[:], pO[:])
            nc.sync.dma_start(d_y[:], yt[:])

    nc.finalize()
    return nc


def _prepare(inputs):
    """Host-side folding of BN, scalings, delays into device weight layouts."""
    f32 = np.float32
    g1 = inputs["bn1_gamma"] / np.sqrt(inputs["bn1_var"] + 1e-5)
    b1 = inputs["bn1_beta"] - inputs["bn1_mean"] * g1
    g2 = inputs["bn2_gamma"] / np.sqrt(inputs["bn2_var"] + 1e-5)
    b2 = inputs["bn2_beta"] - inputs["bn2_mean"] * g2
    al1, rh1, ba1 = inputs["alpha1"], inputs["rho1"], inputs["beta_a1"]
    al2, rh2, ba2 = inputs["alpha2"], inputs["rho2"], inputs["beta_a2"]
    bo = inputs["beta_out"]

    scal = {}
    for k, v in (("al1", al1), ("rh1", rh1), ("ba1", ba1),
                 ("al2", al2), ("rh2", rh2), ("ba2", ba2), ("bo", bo)):
        u = _uniform(v)
        assert u is not None, f"non-uniform {k} not supported by this kernel"
        scal[k] = u
    scal["wt"] = [float((1.0 - scal["bo"] ** (T - t)) / T) for t in range(T)]

    def to_bf16(a):
        return np.ascontiguousarray(a.astype(ml_dtypes.bfloat16))

    use_fp8 = KDR if KV == "2" else FP8
    wd_np = mybir.dt.np(mybir.dt.float8e4) if use_fp8 else ml_dtypes.bfloat16

    def to_wd(a):
        return np.ascontiguousarray(a.astype(wd_np))

    def fold_ktiles(w_eff, nk, m):
        # w_eff: [M, K] effective weight; return lhsT layout [128, nk*m]
        lt = np.ascontiguousarray(w_eff.T)  # [K, M]
        lt = lt.reshape(nk, 128, m).transpose(1, 0, 2).reshape(128, nk * m)
        return lt

    # layer 1 recurrent: fold (1-al)*g and the -(THRESH + beta_a)*I diagonal
    w1 = ((1 - al1) * g1)[:, None].astype(f32) * inputs["W_rec1"]
    w1[np.arange(H1), np.arange(H1)] -= (THRESH + ba1).astype(f32)
    lt1 = to_wd(fold_ktiles(w1, 8, H1))

    w2 = ((1 - al2) * g2)[:, None].astype(f32) * inputs["W2"]
    lt2 = to_wd(fold_ktiles(w2, 8, H2))

    wr2 = ((1 - al2) * g2)[:, None].astype(f32) * inputs["W_rec2"]
    wr2[np.arange(H2), np.arange(H2)] -= (THRESH + ba2).astype(f32)
    ltr2 = to_wd(fold_ktiles(wr2, 4, H2))

    lto = np.ascontiguousarray(
        fold_ktiles(inputs["W_out"].astype(f32), 4, O).astype(f32)
    )

    # input projection with per-channel fractional delays folded into lags
    d = 1.0 / (1.0 + np.exp(-inputs["delay_raw"].astype(np.float64))) * MAX_DELAY
    fl = np.floor(d).astype(np.int64)
    frac = (d - fl).astype(f32)
    lags = sorted(set(fl.tolist()) | set((fl + 1).tolist()))
    n_lags = len(lags)
    assert n_lags <= 2, f"too many distinct delay lags ({n_lags}) for one k-tile"
    # lag block li sits at partitions [64*li, 64*li + C); bias row just after
    # the last lag block, aligned with the device-side ones fill.
    bias_row = 64 if n_lags == 1 else 64 + C
    kin = bias_row + 1
    wd_eff = ((1 - al1) * g1)[:, None].astype(f32) * inputs["W_delay"]  # [H1, C]
    wcat = np.zeros((128, H1), f32)
    for li, lg in enumerate(lags):
        coef = np.where(fl == lg, 1.0 - frac, np.where(fl + 1 == lg, frac, 0.0))
        wcat[64 * li : 64 * li + C, :] = (coef[:, None] * wd_eff.T).astype(f32)
    wcat[bias_row, :] = ((1 - al1) * b1).astype(f32)
    # layer-2 BN bias must be zero for this folding (no per-step bias matmul)
    c2 = (1 - al2) * b2
    assert np.abs(c2).max() == 0.0, "nonzero layer-2 bias not supported"
    wcat_bf = to_bf16(wcat)

    xp = inputs["x"].astype(f32)  # [B, T, C]
    xcores = [
        np.ascontiguousarray(xp[i * BC : (i + 1) * BC].transpose(2, 1, 0))
        for i in range(NCORES)
    ]  # [C, T, BC] each

    common = dict(lt1=lt1, lt2=lt2, ltr2=ltr2, lto=lto, wcat=wcat_bf)
    if KV == "2":
        common["eye"] = np.ascontiguousarray(np.eye(128, dtype=ml_dtypes.bfloat16))
    in_maps = [dict(common, xcore=xcores[i]) for i in range(NCORES)]
    return n_lags, lags, kin, scal, in_maps


def kernel(**inputs):
    global LAST_RESULT
    inputs = {k: np.asarray(v) for k, v in inputs.items()}
    n_lags, lags, kin, scal, in_maps = _prepare(inputs)

    key = (
        KV, KDR, KGPS,
        tuple(lags),
        kin,
        tuple((k, v) for k, v in sorted(scal.items()) if k != "wt"),
        tuple(scal["wt"]),
    )
    nc = _CACHE.get(key)
    if nc is None:
        build = _build_v2 if KV == "2" else _build_bass
        nc = build(n_lags, lags, kin, scal)
        _CACHE[key] = nc

    kw = {}
    if TRACE and TMPDIR:
        os.makedirs(TMPDIR, exist_ok=True)
        kw["tmpdir"] = TMPDIR
    res = run_bass_kernel_spmd(
        nc, in_maps, core_ids=list(range(NCORES)), trace=TRACE, **kw
    )
    LAST_RESULT = res

    out = np.empty((B, O), np.float32)
    for i in range(NCORES):
        out[i * BC : (i + 1) * BC] = res.results[i]["yout"].T
    return out



# revision 6
# speedup vs baseline: 1.0458x; 1.0458x over previous
"""Trainium2 Bass kernel for nn_DelayGSCSNN.

Two-layer adaptive-LIF spiking net with learnable input delays, BN (eval),
and a leaky-integrator readout, scanned over T=100 steps.

Strategy (data-parallel over batch, 8 cores, no collectives):
  - each core simulates B/8 = 32 samples; weights replicated in SBUF.
  - on-device layout: neurons on partitions, batch on the free dim, so the
    spike tiles s1T [128, 8*32] / s2T [128, 4*32] are directly the matmul
    moving operands (rhs) for the next step -- no transposes anywhere.
  - host folds BN (eval stats) and the (1-alpha) input scaling into the
    weight matrices; folds the -THRESH*s and -beta_a*s terms into the
    recurrent weight diagonal; tracks adaptation as A = -a/beta_a so each
    layer's state update is 4 fused DVE ops per step.
  - per-channel fractional delays are folded into a lag-grouped input
    projection (one matmul per step over K = n_lags*C + 1 incl. bias row);
    the lag shift itself is a column-offset copy done once in the prologue.
  - readout: acc = sum_t (1-beta^(T-t))/T * s2_t @ W_out.T, accumulated as
    a weighted spike sum (one DVE op/step) and one matmul at the end.
  - weights in bf16 (spikes are exactly representable; matmul accumulates
    in fp32), state updates in fp32 on the vector engine.
"""

import os
import sys

import numpy as np

for _p in ("/opt/trn_rl_repo", "/root/.axon_site/_ro/trn_rl_repo"):
    if os.path.isdir(_p) and _p not in sys.path:
        sys.path.insert(0, _p)

import concourse.bass as bass
import concourse.tile as tile
from concourse import bacc, mybir
from concourse.bass_utils import run_bass_kernel_spmd

import ml_dtypes

F32 = mybir.dt.float32
BF16 = mybir.dt.bfloat16
OP = mybir.AluOpType

B, T, C = 256, 100, 40
H1, H2, O = 1024, 512, 35
THRESH = 1.0
MAX_DELAY = 30
NCORES = 8
BC = B // NCORES  # batch per core = 32

TRACE = False
TMPDIR = None
LAST_RESULT = None

_CACHE = {}


def _uniform(v):
    v = np.asarray(v, np.float64)
    return float(v.flat[0]) if np.ptp(v) == 0 else None


FP8 = os.environ.get("KFP8", "0") == "1"


def _build_bass(n_lags, lags, kin, scal):
    """Build the Bass program. scal: dict of python-float uniform params."""
    WD = mybir.dt.float8e4 if FP8 else BF16
    nc = bacc.Bacc(None, target_bir_lowering=False)

    # DRAM inputs (per-core shapes; host supplies prepared layouts)
    d_lt1 = nc.dram_tensor("lt1", [128, 8 * H1], WD, kind="ExternalInput")
    d_lt2 = nc.dram_tensor("lt2", [128, 8 * H2], WD, kind="ExternalInput")
    d_ltr2 = nc.dram_tensor("ltr2", [128, 4 * H2], WD, kind="ExternalInput")
    d_lto = nc.dram_tensor("lto", [128, 4 * O], F32, kind="ExternalInput")
    d_wcat = nc.dram_tensor("wcat", [128, H1], BF16, kind="ExternalInput")
    d_x = nc.dram_tensor("xcore", [C, T, BC], F32, kind="ExternalInput")
    d_y = nc.dram_tensor("yout", [O, BC], F32, kind="ExternalOutput")

    A1 = scal["al1"]
    R1 = scal["rh1"]
    RBA1 = scal["rh1"] * scal["ba1"]
    A2 = scal["al2"]
    R2 = scal["rh2"]
    RBA2 = scal["rh2"] * scal["ba2"]
    wt = scal["wt"]  # list of T readout weights (1-beta^(T-t))/T

    with tile.TileContext(nc) as tc:
        with (
            tc.tile_pool(name="const", bufs=1) as cpool,
            tc.tile_pool(name="state", bufs=1) as spool,
            tc.tile_pool(name="psum", bufs=2, space="PSUM") as ppool,
            tc.tile_pool(name="pout", bufs=1, space="PSUM") as opool,
        ):
            # --- weights / input staging ---
            lt1 = cpool.tile([128, 8 * H1], WD)
            lt2 = cpool.tile([128, 8 * H2], WD)
            ltr2 = cpool.tile([128, 4 * H2], WD)
            lto = cpool.tile([128, 4 * O], F32)
            wcat = cpool.tile([128, H1], BF16)
            xf32 = cpool.tile([C, T * BC], F32)
            xt = cpool.tile([128, T * BC], BF16)

            nc.sync.dma_start(lt1[:], d_lt1[:])
            nc.sync.dma_start(lt2[:], d_lt2[:])
            nc.sync.dma_start(ltr2[:], d_ltr2[:])
            nc.sync.dma_start(lto[:], d_lto[:])
            nc.sync.dma_start(wcat[:], d_wcat[:])
            nc.sync.dma_start(xf32[:], d_x[:].rearrange("c t b -> c (t b)"))

            # Engine writes must start at a 32-aligned partition: lag blocks
            # live at partitions 0 and 64; the constant-one bias row is carved
            # from a 32-aligned ones fill (lag1's copy overwrites part of it,
            # leaving row `bias_row` = 1.0 with zero rows after it).
            ones_base = 64 if n_lags == 1 else 96
            salt = len(os.environ.get("KSALT", ""))
            if salt:
                # compile-cache salt: harmless extra memset changes the BIR
                # hash so A/B compiler-flag experiments don't hit the cache
                sc = cpool.tile([1, salt], F32)
                nc.vector.memset(sc[:], 0.0)
            nc.vector.memset(xt[:], 0.0)
            nc.vector.memset(xt[ones_base:128, :], 1.0)
            for li, lg in enumerate(lags):
                if lg < T:
                    nc.vector.tensor_copy(
                        xt[64 * li : 64 * li + C, lg * BC : T * BC],
                        xf32[:, 0 : (T - lg) * BC],
                    )

            # --- states ---
            v1 = spool.tile([128, 8 * BC], F32)
            a1n = spool.tile([128, 8 * BC], F32)
            v2 = spool.tile([128, 4 * BC], F32)
            a2n = spool.tile([128, 4 * BC], F32)
            s2w = spool.tile([128, 4 * BC], F32)
            # spike tiles are parity double-buffered: step t writes buffer
            # t%2 and reads buffer (t-1)%2, so the adaptation update (which
            # reads the OLD spikes) can be emitted AFTER the spike write --
            # keeping the DVE FIFO ahead of the spike write short, since s1
            # gates 96 matmuls of the next round.
            s1b = [spool.tile([128, 8 * BC], BF16, name=f"s1_{i}") for i in range(2)]
            s2b = [spool.tile([128, 4 * BC], BF16, name=f"s2_{i}") for i in range(2)]
            for st in (v1, a1n, v2, a2n, s2w):
                nc.vector.memset(st[:], 0.0)
            for st in s1b + s2b:
                nc.vector.memset(st[:], 0.0)
            if FP8:
                # separate fp8 copies of the spike tiles feed the matmuls
                # (0/1 is exact in fp8); the DVE state update reads bf16.
                s1qb = [spool.tile([128, 8 * BC], WD, name=f"s1q_{i}") for i in range(2)]
                s2qb = [spool.tile([128, 4 * BC], WD, name=f"s2q_{i}") for i in range(2)]
                for st in s1qb + s2qb:
                    nc.vector.memset(st[:], 0.0)
            else:
                s1qb, s2qb = s1b, s2b

            mul, add, sub = OP.mult, OP.add, OP.subtract

            def input_mms(t):
                # input projection into its own PSUM tile as 8 self-contained
                # per-mt groups (start+stop each), so it can be issued one
                # step ahead -- it depends only on xt, and fills the PE stall
                # while the DVE computes the layer-1 spikes.
                pi = ppool.tile([128, 8 * BC], F32, tag="pin")
                for mt in range(8):
                    nc.tensor.matmul(
                        pi[:, mt * BC : (mt + 1) * BC],
                        wcat[0:kin, mt * 128 : (mt + 1) * 128],
                        xt[0:kin, t * BC : (t + 1) * BC],
                        start=True,
                        stop=True,
                    )
                return pi

            pin_next = input_mms(0)

            for t in range(T):
                w, r = t % 2, (t - 1) % 2
                s1, s2 = s1b[w], s2b[w]
                s1o, s2o = s1b[r], s2b[r]
                s1q, s2q = s1qb[w], s2qb[w]
                s1qo, s2qo = s1qb[r], s2qb[r]
                # ---- PE: psum1 = LT1eff @ s1_{t-1} (input part was issued
                # one step ahead into pin_next) ----
                pin = pin_next
                p1 = None
                if t > 0:
                    p1 = ppool.tile([128, 8 * BC], F32, tag="p1")
                    for mt in range(8):
                        po = p1[:, mt * BC : (mt + 1) * BC]
                        for kt in range(8):
                            nc.tensor.matmul(
                                po,
                                lt1[:, kt * H1 + mt * 128 : kt * H1 + (mt + 1) * 128],
                                s1qo[:, kt * BC : (kt + 1) * BC],
                                start=(kt == 0),
                                stop=(kt == 7),
                            )

                p2 = ppool.tile([128, 4 * BC], F32, tag="p2")

                # ---- DVE: layer-1 state update ----
                # v1 = alpha*v1 + pin (runs during rec1) ; v1 += psum1
                # v1 += rho*beta_a*A1neg ; s1 = (v1 >= THRESH)
                # A1neg = rho*A1neg - s1_old (deferred behind the spike write)
                nc.vector.scalar_tensor_tensor(v1[:], v1[:], A1, pin[:], mul, add)
                nc.vector.scalar_tensor_tensor(v1[:], a1n[:], RBA1, v1[:], mul, add)
                if p1 is not None:
                    nc.vector.tensor_add(v1[:], v1[:], p1[:])
                nc.vector.tensor_scalar(s1[:], v1[:], THRESH, None, OP.is_ge)
                if FP8:
                    nc.vector.tensor_scalar(s1q[:], v1[:], THRESH, None, OP.is_ge)
                nc.vector.scalar_tensor_tensor(a1n[:], a1n[:], R1, s1o[:], mul, sub)

                # ---- PE: next step's input projection (independent of s1_t;
                # fills the stall while the DVE computes the spikes) ----
                if t + 1 < T:
                    pin_next = input_mms(t + 1)

                # ---- PE: psum2 = LTr2eff @ s2_{t-1} + LT2eff @ s1_t ----
                # per-mt accumulation groups must be contiguous (one pending
                # group per PSUM tile); rec2 leads so it can start before the
                # DVE finishes s1_t.
                for mt in range(4):
                    po = p2[:, mt * BC : (mt + 1) * BC]
                    if t > 0:
                        for kt in range(4):
                            nc.tensor.matmul(
                                po,
                                ltr2[:, kt * H2 + mt * 128 : kt * H2 + (mt + 1) * 128],
                                s2qo[:, kt * BC : (kt + 1) * BC],
                                start=(kt == 0),
                                stop=False,
                            )
                    for kt in range(8):
                        nc.tensor.matmul(
                            po,
                            lt2[:, kt * H2 + mt * 128 : kt * H2 + (mt + 1) * 128],
                            s1q[:, kt * BC : (kt + 1) * BC],
                            start=(kt == 0 and t == 0),
                            stop=(kt == 7),
                        )

                # ---- DVE: layer-2 state update + readout accumulation ----
                nc.vector.scalar_tensor_tensor(v2[:], v2[:], A2, p2[:], mul, add)
                nc.vector.scalar_tensor_tensor(v2[:], a2n[:], RBA2, v2[:], mul, add)
                nc.vector.tensor_scalar(s2[:], v2[:], THRESH, None, OP.is_ge)
                if FP8:
                    nc.vector.tensor_scalar(s2q[:], v2[:], THRESH, None, OP.is_ge)
                nc.vector.scalar_tensor_tensor(a2n[:], a2n[:], R2, s2o[:], mul, sub)
                nc.vector.scalar_tensor_tensor(s2w[:], s2[:], wt[t], s2w[:], mul, add)

            # ---- epilogue: acc^T = W_out @ s2w  -> DMA out ----
            pO = opool.tile([O, BC], F32)
            # matmul needs matching dtypes; s2w is fp32, lto fp32 (4 cyc/row,
            # only 4 small matmuls).
            for kt in range(4):
                nc.tensor.matmul(
                    pO[:],
                    lto[:, kt * O : (kt + 1) * O],
                    s2w[:, kt * BC : (kt + 1) * BC],
                    start=(kt == 0),
                    stop=(kt == 3),
                )
            yt = spool.tile([O, BC], F32)
            nc.vector.tensor_copy(yt[:], pO[:])
            nc.sync.dma_start(d_y[:], yt[:])

    nc.finalize()
    return nc


def _build_v3(n_lags, lags, kin, scal):
    """v3: threshold-form state + lagged layer 2 + ACT/Pool offload.

    Math: per layer track w(t) = TH - alpha*v(t-1) - rho*beta_a*k(t-1) [- pin(t)]
    so the spike is s(t) = (p_rec(t) >= w(t)) -- a single DVE tensor_tensor
    is_ge reading PSUM directly.  All bookkeeping:
      w(t+1) = TH*(1-alpha) + alpha*w(t) - alpha*p_rec(t) - RBA*k(t) - pin(t+1)
      kR(t)  = R*kR(t-1) - RBA*s(t-1)        (kR = RBA * adaptation state)
    runs off the critical path on the Scalar (ACT) and Pool (gpsimd) engines:
      ACT:  q = Copy(p_rec * -alpha + TH*(1-alpha)); sR = Copy(s_old * RBA);
            kRs = Copy(kR * R)
      Pool: kR = kRs - sR;  w = ((w*alpha)+q [DVE stt]) - kR - pin_next
    Layer 2 lags one step behind layer 1 in the PE stream, so every matmul
    consumes spikes that are >= 1 full step old: the PE never waits on the
    DVE/ACT/Pool mid-stream, which keeps the issue rate dense and HAM warm.
    """
    nc = bacc.Bacc(None, target_bir_lowering=False)

    d_lt1 = nc.dram_tensor("lt1", [128, 8 * H1], BF16, kind="ExternalInput")
    d_lt2 = nc.dram_tensor("lt2", [128, 8 * H2], BF16, kind="ExternalInput")
    d_ltr2 = nc.dram_tensor("ltr2", [128, 4 * H2], BF16, kind="ExternalInput")
    d_lto = nc.dram_tensor("lto", [128, 4 * O], F32, kind="ExternalInput")
    d_wcat = nc.dram_tensor("wcat", [128, H1], BF16, kind="ExternalInput")
    d_x = nc.dram_tensor("xcore", [C, T, BC], F32, kind="ExternalInput")
    d_y = nc.dram_tensor("yout", [O, BC], F32, kind="ExternalOutput")

    A1, R1, RBA1 = scal["al1"], scal["rh1"], scal["rh1"] * scal["ba1"]
    A2, R2, RBA2 = scal["al2"], scal["rh2"], scal["rh2"] * scal["ba2"]
    C1 = THRESH * (1.0 - A1)
    C2 = THRESH * (1.0 - A2)
    wt = scal["wt"]
    mul, add, sub = OP.mult, OP.add, OP.subtract
    Copy = mybir.ActivationFunctionType.Copy

    with tile.TileContext(nc) as tc:
        with (
            tc.tile_pool(name="const", bufs=1) as cpool,
            tc.tile_pool(name="state", bufs=1) as spool,
            tc.tile_pool(name="psum", bufs=2, space="PSUM") as ppool,
            tc.tile_pool(name="pout", bufs=1, space="PSUM") as opool,
        ):
            lt1 = cpool.tile([128, 8 * H1], BF16)
            lt2 = cpool.tile([128, 8 * H2], BF16)
            ltr2 = cpool.tile([128, 4 * H2], BF16)
            lto = cpool.tile([128, 4 * O], F32)
            wcat = cpool.tile([128, H1], BF16)
            xf32 = cpool.tile([C, T * BC], F32)
            xt = cpool.tile([128, T * BC], BF16)

            nc.sync.dma_start(lt1[:], d_lt1[:])
            nc.sync.dma_start(lt2[:], d_lt2[:])
            nc.sync.dma_start(ltr2[:], d_ltr2[:])
            nc.sync.dma_start(lto[:], d_lto[:])
            nc.sync.dma_start(wcat[:], d_wcat[:])
            nc.sync.dma_start(xf32[:], d_x[:].rearrange("c t b -> c (t b)"))

            ones_base = 64 if n_lags == 1 else 96
            salt = len(os.environ.get("KSALT", ""))
            if salt:
                sc = cpool.tile([1, salt], F32)
                nc.vector.memset(sc[:], 0.0)
            nc.vector.memset(xt[:], 0.0)
            nc.vector.memset(xt[ones_base:128, :], 1.0)
            for li, lg in enumerate(lags):
                if lg < T:
                    nc.vector.tensor_copy(
                        xt[64 * li : 64 * li + C, lg * BC : T * BC],
                        xf32[:, 0 : (T - lg) * BC],
                    )

            # --- states ---
            w1 = spool.tile([128, 8 * BC], F32)
            w2 = spool.tile([128, 4 * BC], F32)
            kR1 = spool.tile([128, 8 * BC], F32)
            kR2 = spool.tile([128, 4 * BC], F32)
            sR1 = spool.tile([128, 8 * BC], F32)
            sR2 = spool.tile([128, 4 * BC], F32)
            kRs1 = spool.tile([128, 8 * BC], F32)
            kRs2 = spool.tile([128, 4 * BC], F32)
            q1 = spool.tile([128, 8 * BC], F32)
            q2 = spool.tile([128, 4 * BC], F32)
            wtmp1 = spool.tile([128, 8 * BC], F32)
            wtmp2 = spool.tile([128, 4 * BC], F32)
            s2ws = spool.tile([128, 4 * BC], F32)
            s2w = spool.tile([128, 4 * BC], F32)
            s1b = [spool.tile([128, 8 * BC], BF16, name=f"s1_{i}") for i in range(2)]
            s2b = [spool.tile([128, 4 * BC], BF16, name=f"s2_{i}") for i in range(2)]
            nc.vector.memset(w1[:], THRESH)
            nc.vector.memset(w2[:], THRESH)
            for st in (kR1, kR2, s2w):
                nc.vector.memset(st[:], 0.0)
            for st in s1b + s2b:
                nc.vector.memset(st[:], 0.0)

            def pin_mms(t):
                # input projection for step t: 8 self-contained per-mt groups
                pi = ppool.tile([128, 8 * BC], F32, tag="pin")
                for mt in range(8):
                    nc.tensor.matmul(
                        pi[:, mt * BC : (mt + 1) * BC],
                        wcat[0:kin, mt * 128 : (mt + 1) * 128],
                        xt[0:kin, t * BC : (t + 1) * BC],
                        start=True,
                        stop=True,
                    )
                return pi

            # prologue: pin(0); fold it into w1(0) = TH - pin(0)
            pin_cur = pin_mms(0)
            nc.vector.scalar_tensor_tensor(w1[:], pin_cur[:], -1.0, w1[:], mul, add)

            for t in range(T):
                j = t - 1
                w, r = t % 2, (t - 1) % 2
                s1, s1o = s1b[w], s1b[r]

                # aux: adaptation chain for layer 1 (uses s1(t-1); feeds w1(t+1))
                if t + 1 < T:
                    nc.scalar.activation(sR1[:], s1o[:], Copy, scale=RBA1)
                    nc.scalar.activation(kRs1[:], kR1[:], Copy, scale=R1)
                    nc.gpsimd.tensor_sub(kR1[:], kRs1[:], sR1[:])

                # ---- PE: p1(t) = Wrec1_eff @ s1(t-1), 8 per-mt groups ----
                p1 = ppool.tile([128, 8 * BC], F32, tag="p1")
                for mt in range(8):
                    po = p1[:, mt * BC : (mt + 1) * BC]
                    for kt in range(8):
                        nc.tensor.matmul(
                            po,
                            lt1[:, kt * H1 + mt * 128 : kt * H1 + (mt + 1) * 128],
                            s1o[:, kt * BC : (kt + 1) * BC],
                            start=(kt == 0),
                            stop=(kt == 7),
                        )

                # ---- DVE: spike (the only critical cross-engine op) ----
                nc.vector.tensor_tensor(s1[:], p1[:], w1[:], OP.is_ge)
                # ---- ACT: q1(t) = -A1*p1 + TH*(1-A1) ----
                if t + 1 < T:
                    nc.scalar.activation(q1[:], p1[:], Copy, scale=-A1, bias=C1)

                # ---- PE: pin(t+1) (filler; feeds w1(t+1) via Pool) ----
                pin_next = pin_mms(t + 1) if t + 1 < T else None

                # ---- PE: layer 2 for step j = t-1 ----
                if j >= 0:
                    s2, s2o = s2b[j % 2], s2b[(j - 1) % 2]
                    s1j = s1b[j % 2]
                    p2 = ppool.tile([128, 4 * BC], F32, tag="p2")
                    for mt in range(4):
                        po = p2[:, mt * BC : (mt + 1) * BC]
                        if j > 0:
                            for kt in range(4):
                                nc.tensor.matmul(
                                    po,
                                    ltr2[:, kt * H2 + mt * 128 : kt * H2 + (mt + 1) * 128],
                                    s2o[:, kt * BC : (kt + 1) * BC],
                                    start=(kt == 0),
                                    stop=False,
                                )
                        for kt in range(8):
                            nc.tensor.matmul(
                                po,
                                lt2[:, kt * H2 + mt * 128 : kt * H2 + (mt + 1) * 128],
                                s1j[:, kt * BC : (kt + 1) * BC],
                                start=(kt == 0 and j == 0),
                                stop=(kt == 7),
                            )
                    nc.vector.tensor_tensor(s2[:], p2[:], w2[:], OP.is_ge)
                    if j + 1 < T:
                        nc.scalar.activation(q2[:], p2[:], Copy, scale=-A2, bias=C2)

                # ---- DVE+Pool: w1(t+1) ----
                if t + 1 < T:
                    nc.vector.scalar_tensor_tensor(wtmp1[:], w1[:], A1, q1[:], mul, add)
                    nc.gpsimd.tensor_sub(wtmp1[:], wtmp1[:], kR1[:])
                    nc.vector.scalar_tensor_tensor(
                        w1[:], pin_next[:], -1.0, wtmp1[:], mul, add
                    )

                # ---- aux: layer-2 chains ----
                if j >= 0 and j + 1 < T:
                    nc.scalar.activation(sR2[:], s2o[:], Copy, scale=RBA2)
                    nc.scalar.activation(kRs2[:], kR2[:], Copy, scale=R2)
                    nc.gpsimd.tensor_sub(kR2[:], kRs2[:], sR2[:])
                    nc.vector.scalar_tensor_tensor(wtmp2[:], w2[:], A2, q2[:], mul, add)
                    nc.gpsimd.tensor_sub(w2[:], wtmp2[:], kR2[:])
                if j >= 0:
                    # readout accumulation: s2w += wt[j] * s2(j)
                    nc.scalar.activation(s2ws[:], s2b[j % 2][:], Copy, scale=wt[j])
                    nc.gpsimd.tensor_add(s2w[:], s2w[:], s2ws[:])

            # ---- drain: layer 2 for j = T-1 ----
            j = T - 1
            s2, s2o = s2b[j % 2], s2b[(j - 1) % 2]
            s1j = s1b[j % 2]
            p2 = ppool.tile([128, 4 * BC], F32, tag="p2")
            for mt in range(4):
                po = p2[:, mt * BC : (mt + 1) * BC]
                for kt in range(4):
                    nc.tensor.matmul(
                        po,
                        ltr2[:, kt * H2 + mt * 128 : kt * H2 + (mt + 1) * 128],
                        s2o[:, kt * BC : (kt + 1) * BC],
                        start=(kt == 0),
                        stop=False,
                    )
                for kt in range(8):
                    nc.tensor.matmul(
                        po,
                        lt2[:, kt * H2 + mt * 128 : kt * H2 + (mt + 1) * 128],
                        s1j[:, kt * BC : (kt + 1) * BC],
                        start=False,
                        stop=(kt == 7),
                    )
            nc.vector.tensor_tensor(s2[:], p2[:], w2[:], OP.is_ge)
            nc.scalar.activation(s2ws[:], s2[:], Copy, scale=wt[j])
            nc.gpsimd.tensor_add(s2w[:], s2w[:], s2ws[:])

            # ---- epilogue: acc^T = W_out @ s2w -> DMA out ----
            pO = opool.tile([O, BC], F32)
            for kt in range(4):
                nc.tensor.matmul(
                    pO[:],
                    lto[:, kt * O : (kt + 1) * O],
                    s2w[:, kt * BC : (kt + 1) * BC],
                    start=(kt == 0),
                    stop=(kt == 3),
                )
            yt = spool.tile([O, BC], F32)
            nc.vector.tensor_copy(yt[:], pO[:])
            nc.sync.dma_start(d_y[:], yt[:])

    nc.finalize()
    return nc


# KV=1: original build (p1 separate, DVE-heavy, ~537us measured).
# KV=2: experimental PSUM-injection build -- neutral vs v1 (~548us): a
#   ~1.4us/step cross-engine semaphore serialization ate the pipelining win.
# KV=3 (default): threshold-form + lagged layer 2 + ACT/Pool offload.
KV = os.environ.get("KV", "3")
KDR = os.environ.get("KDR", "0") == "1"  # fp8 DoubleRow for rec/ff matmuls
# (measured: DoubleRow LDWEIGHTS is ~3x slower per tile on this hw -- keep off)
KGPS = os.environ.get("KGPS", "0") == "1"  # adaptation/m updates on GpSimd
# (GpSimd lowers to the Pool engine on TRN2, which rejects TensorScalarPtr
# at codegen -- keep these on the DVE.)


def _build_v2(n_lags, lags, kin, scal):
    """v2: all linear state terms injected into the PSUM accumulation group
    (identity matmul on m = alpha*v + kRBA*chat), spike threshold is the only
    critical-path DVE op, layer-2 matmuls lag one step behind layer 1 in the
    PE stream so the PE never waits on the spike DVE op. ACT does the
    leak-scaled PSUM->SBUF copies, GpSimd the adaptation updates.
    Spikes stored as exact 0/1 (fp8 when KDR, else bf16)."""
    WD = mybir.dt.float8e4 if KDR else BF16
    nc = bacc.Bacc(None, target_bir_lowering=False)

    d_lt1 = nc.dram_tensor("lt1", [128, 8 * H1], WD, kind="ExternalInput")
    d_lt2 = nc.dram_tensor("lt2", [128, 8 * H2], WD, kind="ExternalInput")
    d_ltr2 = nc.dram_tensor("ltr2", [128, 4 * H2], WD, kind="ExternalInput")
    d_lto = nc.dram_tensor("lto", [128, 4 * O], F32, kind="ExternalInput")
    d_wcat = nc.dram_tensor("wcat", [128, H1], BF16, kind="ExternalInput")
    d_eye = nc.dram_tensor("eye", [128, 128], BF16, kind="ExternalInput")
    d_x = nc.dram_tensor("xcore", [C, T, BC], F32, kind="ExternalInput")
    d_y = nc.dram_tensor("yout", [O, BC], F32, kind="ExternalOutput")

    A1, R1, RBA1 = scal["al1"], scal["rh1"], scal["rh1"] * scal["ba1"]
    A2, R2, RBA2 = scal["al2"], scal["rh2"], scal["rh2"] * scal["ba2"]
    wt = scal["wt"]

    with tile.TileContext(nc) as tc:
        with (
            tc.tile_pool(name="const", bufs=1) as cpool,
            tc.tile_pool(name="state", bufs=1) as spool,
            tc.tile_pool(name="psum1", bufs=2, space="PSUM") as p1pool,
            tc.tile_pool(name="psum2", bufs=2, space="PSUM") as p2pool,
            tc.tile_pool(name="pout", bufs=1, space="PSUM") as opool,
        ):
            lt1 = cpool.tile([128, 8, H1], WD)
            lt2 = cpool.tile([128, 8, H2], WD)
            ltr2 = cpool.tile([128, 4, H2], WD)
            lto = cpool.tile([128, 4 * O], F32)
            wcat = cpool.tile([128, H1], BF16)
            eye = cpool.tile([128, 128], BF16)
            xf32 = cpool.tile([C, T * BC], F32)
            xt = cpool.tile([128, T * BC], BF16)

            nc.sync.dma_start(lt1[:], d_lt1[:].rearrange("p (k m) -> p k m", k=8))
            nc.sync.dma_start(lt2[:], d_lt2[:].rearrange("p (k m) -> p k m", k=8))
            nc.sync.dma_start(ltr2[:], d_ltr2[:].rearrange("p (k m) -> p k m", k=4))
            nc.sync.dma_start(lto[:], d_lto[:])
            nc.sync.dma_start(wcat[:], d_wcat[:])
            nc.sync.dma_start(eye[:], d_eye[:])
            nc.sync.dma_start(xf32[:], d_x[:].rearrange("c t b -> c (t b)"))

            ones_base = 64 if n_lags == 1 else 96
            nc.vector.memset(xt[:], 0.0)
            nc.vector.memset(xt[ones_base:128, :], 1.0)
            for li, lg in enumerate(lags):
                if lg < T:
                    nc.vector.tensor_copy(
                        xt[64 * li : 64 * li + C, lg * BC : T * BC],
                        xf32[:, 0 : (T - lg) * BC],
                    )

            # --- states (3D: [128, chunk, batch]) ---
            s1b = [spool.tile([128, 8, BC], WD, name=f"s1_{i}") for i in range(2)]
            s2b = [spool.tile([128, 4, BC], WD, name=f"s2_{i}") for i in range(2)]
            c1 = spool.tile([128, 8, BC], BF16)
            c2 = spool.tile([128, 4, BC], BF16)
            va1b = [spool.tile([128, 8, BC], BF16, name=f"va1_{i}") for i in range(2)]
            va2b = [spool.tile([128, 4, BC], BF16, name=f"va2_{i}") for i in range(2)]
            m1b = [spool.tile([128, 8, BC], BF16, name=f"m1_{i}") for i in range(2)]
            m2b = [spool.tile([128, 4, BC], BF16, name=f"m2_{i}") for i in range(2)]
            s2w = spool.tile([128, 4, BC], F32)
            for st in s1b + s2b + va1b + va2b + m1b + m2b + [c1, c2, s2w]:
                nc.vector.memset(st[:], 0.0)

            mul, add, sub = OP.mult, OP.add, OP.subtract
            aux = nc.gpsimd if KGPS else nc.vector

            def g1(i):
                # layer-1 accumulation group for step i into p1 (per-mt
                # contiguous groups: pin opens, rec1 accumulates, ident(m1)
                # closes).  p1 = pin(i) + W1eff@s1(i-1) + m1(i-1)
                # All states zero-initialized, so every step is uniform.
                p1 = p1pool.tile([128, 8, BC], F32, tag="p1")
                s1o = s1b[(i - 1) % 2]
                m1 = m1b[(i - 1) % 2]
                for mt in range(8):
                    po = p1[:, mt, :]
                    ms = slice(mt * 128, (mt + 1) * 128)
                    nc.tensor.matmul(
                        po,
                        wcat[0:kin, ms],
                        xt[0:kin, i * BC : (i + 1) * BC],
                        start=True,
                        stop=False,
                    )
                    if KDR:
                        for kp in range(4):
                            nc.tensor.matmul(
                                po, lt1[:, 2 * kp : 2 * kp + 2, ms],
                                s1o[:, 2 * kp : 2 * kp + 2, :],
                                start=False, stop=False,
                                perf_mode=mybir.MatmulPerfMode.DoubleRow,
                            )
                    else:
                        for kt in range(8):
                            nc.tensor.matmul(
                                po, lt1[:, kt, ms], s1o[:, kt, :],
                                start=False, stop=False,
                            )
                    nc.tensor.matmul(po, eye[:], m1[:, mt, :], start=False, stop=True)
                return p1

            def g2(j):
                # layer-2 group for step j: p2 = W2eff@s1(j) + Wr2eff@s2(j-1)
                # + m2(j-1)
                p2 = p2pool.tile([128, 4, BC], F32, tag="p2")
                s1c = s1b[j % 2]
                s2o = s2b[(j - 1) % 2]
                m2 = m2b[(j - 1) % 2]
                for mt in range(4):
                    po = p2[:, mt, :]
                    ms = slice(mt * 128, (mt + 1) * 128)
                    if KDR:
                        for kp in range(4):
                            nc.tensor.matmul(
                                po, lt2[:, 2 * kp : 2 * kp + 2, ms],
                                s1c[:, 2 * kp : 2 * kp + 2, :],
                                start=(kp == 0), stop=False,
                                perf_mode=mybir.MatmulPerfMode.DoubleRow,
                            )
                        for kp in range(2):
                            nc.tensor.matmul(
                                po, ltr2[:, 2 * kp : 2 * kp + 2, ms],
                                s2o[:, 2 * kp : 2 * kp + 2, :],
                                start=False, stop=False,
                                perf_mode=mybir.MatmulPerfMode.DoubleRow,
                            )
                    else:
                        for kt in range(8):
                            nc.tensor.matmul(
                                po, lt2[:, kt, ms], s1c[:, kt, :],
                                start=(kt == 0), stop=False,
                            )
                        for kt in range(4):
                            nc.tensor.matmul(
                                po, ltr2[:, kt, ms], s2o[:, kt, :],
                                start=False, stop=False,
                            )
                    nc.tensor.matmul(po, eye[:], m2[:, mt, :], start=False, stop=True)
                return p2

            Copy = mybir.ActivationFunctionType.Copy
            p2b = [None, None]
            for i in range(T):
                j = i - 1
                p1_new = g1(i)
                # DVE: layer-1 spikes for step i (the only critical-path op)
                # -- emitted immediately after its producing group so Tile's
                # tick-based waits release it as early as possible.
                nc.vector.tensor_scalar(
                    s1b[i % 2][:], p1_new[:], THRESH, None, OP.is_ge
                )
                # ACT: leak-scaled copy valpha1(i) = A1 * v1(i)
                nc.scalar.activation(va1b[i % 2][:], p1_new[:], Copy, scale=A1)

                # DVE: is_ge2 for step j-1 is emitted one iteration LATE
                # (before g2(j), which consumes its spikes): its conservative
                # PE sem wait is then already satisfied when it executes, so
                # the critical is_ge1 above never queues behind it.
                if i >= 2:
                    nc.vector.tensor_scalar(
                        s2b[(j - 1) % 2][:], p2b[(j - 1) % 2][:], THRESH, None,
                        OP.is_ge,
                    )
                p2_new = g2(j) if i > 0 else None
                if p2_new is not None:
                    nc.scalar.activation(va2b[j % 2][:], p2_new[:], Copy, scale=A2)
                    p2b[j % 2] = p2_new

                # DVE: adaptation + m updates (off critical path)
                if i >= 1:
                    # chat1(i) = R1*chat1(i-1) - s1(i-1)
                    aux.scalar_tensor_tensor(
                        c1[:], c1[:], R1, s1b[(i - 1) % 2][:], mul, sub
                    )
                # m1(i) = RBA1*chat1(i) + valpha1(i)
                aux.scalar_tensor_tensor(
                    m1b[i % 2][:], c1[:], RBA1, va1b[i % 2][:], mul, add
                )
                if p2_new is not None:
                    if j >= 1:
                        aux.scalar_tensor_tensor(
                            c2[:], c2[:], R2, s2b[(j - 1) % 2][:], mul, sub
                        )
                    aux.scalar_tensor_tensor(
                        m2b[j % 2][:], c2[:], RBA2, va2b[j % 2][:], mul, add
                    )
                    if j >= 1:
                        # readout accumulation for step j-1
                        aux.scalar_tensor_tensor(
                            s2w[:], s2b[(j - 1) % 2][:], wt[j - 1], s2w[:], mul, add
                        )

            # drain: spikes/readout for steps T-2 and T-1 of layer 2
            j = T - 1
            p2_new = g2(j)
            p2b[j % 2] = p2_new
            nc.vector.tensor_scalar(
                s2b[(j - 1) % 2][:], p2b[(j - 1) % 2][:], THRESH, None, OP.is_ge
            )
            aux.scalar_tensor_tensor(
                s2w[:], s2b[(j - 1) % 2][:], wt[j - 1], s2w[:], mul, add
            )
            nc.vector.tensor_scalar(s2b[j % 2][:], p2_new[:], THRESH, None, OP.is_ge)
            aux.scalar_tensor_tensor(s2w[:], s2b[j % 2][:], wt[j], s2w[:], mul, add)

            # ---- epilogue: acc^T = W_out @ s2w -> DMA out ----
            pO = opool.tile([O, BC], F32)
            for kt in range(4):
                nc.tensor.matmul(
                    pO[:],
                    lto[:, kt * O : (kt + 1) * O],
                    s2w[:, kt, :],
                    start=(kt == 0),
                    stop=(kt == 3),
                )
            yt = spool.tile([O, BC], F32)
            nc.vector.tensor_copy(yt[:], pO[:])
            nc.sync.dma_start(d_y[:], yt[:])

    nc.finalize()
    return nc


def _prepare(inputs):
    """Host-side folding of BN, scalings, delays into device weight layouts."""
    f32 = np.float32
    g1 = inputs["bn1_gamma"] / np.sqrt(inputs["bn1_var"] + 1e-5)
    b1 = inputs["bn1_beta"] - inputs["bn1_mean"] * g1
    g2 = inputs["bn2_gamma"] / np.sqrt(inputs["bn2_var"] + 1e-5)
    b2 = inputs["bn2_beta"] - inputs["bn2_mean"] * g2
    al1, rh1, ba1 = inputs["alpha1"], inputs["rho1"], inputs["beta_a1"]
    al2, rh2, ba2 = inputs["alpha2"], inputs["rho2"], inputs["beta_a2"]
    bo = inputs["beta_out"]

    scal = {}
    for k, v in (("al1", al1), ("rh1", rh1), ("ba1", ba1),
                 ("al2", al2), ("rh2", rh2), ("ba2", ba2), ("bo", bo)):
        u = _uniform(v)
        assert u is not None, f"non-uniform {k} not supported by this kernel"
        scal[k] = u
    scal["wt"] = [float((1.0 - scal["bo"] ** (T - t)) / T) for t in range(T)]

    def to_bf16(a):
        return np.ascontiguousarray(a.astype(ml_dtypes.bfloat16))

    use_fp8 = KDR if KV == "2" else FP8
    wd_np = mybir.dt.np(mybir.dt.float8e4) if use_fp8 else ml_dtypes.bfloat16

    def to_wd(a):
        return np.ascontiguousarray(a.astype(wd_np))

    def fold_ktiles(w_eff, nk, m):
        # w_eff: [M, K] effective weight; return lhsT layout [128, nk*m]
        lt = np.ascontiguousarray(w_eff.T)  # [K, M]
        lt = lt.reshape(nk, 128, m).transpose(1, 0, 2).reshape(128, nk * m)
        return lt

    # layer 1 recurrent: fold (1-al)*g and the -(THRESH + beta_a)*I diagonal
    w1 = ((1 - al1) * g1)[:, None].astype(f32) * inputs["W_rec1"]
    w1[np.arange(H1), np.arange(H1)] -= (THRESH + ba1).astype(f32)
    lt1 = to_wd(fold_ktiles(w1, 8, H1))

    w2 = ((1 - al2) * g2)[:, None].astype(f32) * inputs["W2"]
    lt2 = to_wd(fold_ktiles(w2, 8, H2))

    wr2 = ((1 - al2) * g2)[:, None].astype(f32) * inputs["W_rec2"]
    wr2[np.arange(H2), np.arange(H2)] -= (THRESH + ba2).astype(f32)
    ltr2 = to_wd(fold_ktiles(wr2, 4, H2))

    lto = np.ascontiguousarray(
        fold_ktiles(inputs["W_out"].astype(f32), 4, O).astype(f32)
    )

    # input projection with per-channel fractional delays folded into lags
    d = 1.0 / (1.0 + np.exp(-inputs["delay_raw"].astype(np.float64))) * MAX_DELAY
    fl = np.floor(d).astype(np.int64)
    frac = (d - fl).astype(f32)
    lags = sorted(set(fl.tolist()) | set((fl + 1).tolist()))
    n_lags = len(lags)
    assert n_lags <= 2, f"too many distinct delay lags ({n_lags}) for one k-tile"
    # lag block li sits at partitions [64*li, 64*li + C); bias row just after
    # the last lag block, aligned with the device-side ones fill.
    bias_row = 64 if n_lags == 1 else 64 + C
    kin = bias_row + 1
    wd_eff = ((1 - al1) * g1)[:, None].astype(f32) * inputs["W_delay"]  # [H1, C]
    wcat = np.zeros((128, H1), f32)
    for li, lg in enumerate(lags):
        coef = np.where(fl == lg, 1.0 - frac, np.where(fl + 1 == lg, frac, 0.0))
        wcat[64 * li : 64 * li + C, :] = (coef[:, None] * wd_eff.T).astype(f32)
    wcat[bias_row, :] = ((1 - al1) * b1).astype(f32)
    # layer-2 BN bias must be zero for this folding (no per-step bias matmul)
    c2 = (1 - al2) * b2
    assert np.abs(c2).max() == 0.0, "nonzero layer-2 bias not supported"
    wcat_bf = to_bf16(wcat)

    xp = inputs["x"].astype(f32)  # [B, T, C]
    xcores = [
        np.ascontiguousarray(xp[i * BC : (i + 1) * BC].transpose(2, 1, 0))
        for i in range(NCORES)
    ]  # [C, T, BC] each

    common = dict(lt1=lt1, lt2=lt2, ltr2=ltr2, lto=lto, wcat=wcat_bf)
    if KV == "2":
        common["eye"] = np.ascontiguousarray(np.eye(128, dtype=ml_dtypes.bfloat16))
    in_maps = [dict(common, xcore=xcores[i]) for i in range(NCORES)]
    return n_lags, lags, kin, scal, in_maps


def kernel(**inputs):
    global LAST_RESULT
    inputs = {k: np.asarray(v) for k, v in inputs.items()}
    n_lags, lags, kin, scal, in_maps = _prepare(inputs)

    key = (
        KV, KDR, KGPS,
        tuple(lags),
        kin,
        tuple((k, v) for k, v in sorted(scal.items()) if k != "wt"),
        tuple(scal["wt"]),
    )
    nc = _CACHE.get(key)
    if nc is None:
        build = {"2": _build_v2, "3": _build_v3}.get(KV, _build_bass)
        nc = build(n_lags, lags, kin, scal)
        _CACHE[key] = nc

    kw = {}
    if TRACE and TMPDIR:
        os.makedirs(TMPDIR, exist_ok=True)
        kw["tmpdir"] = TMPDIR
    res = run_bass_kernel_spmd(
        nc, in_maps, core_ids=list(range(NCORES)), trace=TRACE, **kw
    )
    LAST_RESULT = res

    out = np.empty((B, O), np.float32)
    for i in range(NCORES):
        out[i * BC : (i + 1) * BC] = res.results[i]["yout"].T
    return out

